# revision 13
# baseline (speedup 1.0000x reference)
import sys

if "/opt/trn_rl_repo" not in sys.path:
    sys.path.insert(0, "/opt/trn_rl_repo")

from contextlib import ExitStack

import numpy as np

import concourse.bass as bass
import concourse.tile as tile
from concourse import masks, mybir
from concourse.bacc import Bacc
from concourse.bass_utils import run_bass_kernel_spmd

B, S, D, H, HD = 2, 2048, 1024, 16, 64
NCORES = 8
GH = 4                # heads per core
NPAIR = 2             # head pairs per core
ET = D // 128         # 8 contraction tiles over embedding dim
KTN = S // 128        # 16 key tiles
QB = S // 512         # 4 query blocks

F32 = mybir.dt.float32
F16 = mybir.dt.float16
F32R = mybir.dt.float32r
AF = mybir.ActivationFunctionType

_prog = None


def _build():
    nc = Bacc()
    xT_d = nc.declare_dram_parameter("xT", [ET, 128, S], F16, isOutput=False)
    wqk_d = nc.declare_dram_parameter("wqk", [ET, 128, 512], F16, isOutput=False)
    wv_d = nc.declare_dram_parameter("wv", [ET, 128, 256], F16, isOutput=False)
    wo_d = nc.declare_dram_parameter("wo", [2, 128, 1024], F16, isOutput=False)
    bqk_d = nc.declare_dram_parameter("bqk", [128, 4], F32, isOutput=False)
    out_d = nc.declare_dram_parameter("out", [S, D], F32, isOutput=True)

    with tile.TileContext(nc) as tc, ExitStack() as ctx:
        consts = ctx.enter_context(tc.tile_pool(name="consts", bufs=1))
        persist = ctx.enter_context(tc.tile_pool(name="persist", bufs=1))

        bias_sb = consts.tile([128, 4], F32, tag="bias", name="bias_sb")
        nc.sync.dma_start(out=bias_sb, in_=bqk_d[:])
        ident = consts.tile([128, 128], F16, tag="ident", name="ident")
        masks.make_identity(nc, ident)
        wo_sb = consts.tile([128, 2, 1024], F16, tag="wo", name="wo_sb")
        for j in range(2):
            nc.sync.dma_start(out=wo_sb[:, j, :], in_=wo_d[j])

        QTs = [persist.tile([128, S], F16, tag=f"qt{p}", name=f"qt{p}")
               for p in range(NPAIR)]
        KTs = [persist.tile([128, S], F16, tag=f"kt{p}", name=f"kt{p}")
               for p in range(NPAIR)]
        Vones = [persist.tile([128, GH, 65], F16, tag=f"v{t}", name=f"v{t}")
                 for t in range(KTN)]
        OTs = [persist.tile([128, S], F16, tag=f"ot{p}", name=f"ot{p}")
               for p in range(NPAIR)]

        # ---- projection phase: Q/K (pairs packed on partitions) and V ----
        with tc.tile_pool(name="projsb", bufs=1) as pj_sb, \
             tc.tile_pool(name="projps", bufs=2, space="PSUM") as pj_ps:
            xT_sb = pj_sb.tile([128, ET, S], F16, tag="xt", name="xT_sb")
            for et in range(ET):
                nc.sync.dma_start(out=xT_sb[:, et, :], in_=xT_d[et])
            wqk_sb = pj_sb.tile([128, ET, 512], F16, tag="wqk", name="wqk_sb")
            for et in range(ET):
                nc.sync.dma_start(out=wqk_sb[:, et, :], in_=wqk_d[et])
            wv_sb = pj_sb.tile([128, ET, 256], F16, tag="wv", name="wv_sb")
            for et in range(ET):
                nc.sync.dma_start(out=wv_sb[:, et, :], in_=wv_d[et])

            for p in range(NPAIR):
                for qk in range(2):
                    dst = QTs[p] if qk == 0 else KTs[p]
                    col = 2 * p + qk
                    for sb_i in range(QB):
                        ps = pj_ps.tile([128, 512], F32, tag="pj", name="ps_qk")
                        for et in range(ET):
                            nc.tensor.matmul(
                                ps,
                                lhsT=wqk_sb[:, et, col * 128:(col + 1) * 128],
                                rhs=xT_sb[:, et, sb_i * 512:(sb_i + 1) * 512],
                                start=(et == 0), stop=(et == ET - 1),
                            )
                        nc.vector.tensor_scalar_add(
                            dst[:, sb_i * 512:(sb_i + 1) * 512], ps,
                            bias_sb[:, col:col + 1],
                        )

            for st in range(KTN):
                psv = pj_ps.tile([128, 256], F32, tag="pv", name="ps_v")
                for et in range(ET):
                    nc.tensor.matmul(
                        psv,
                        lhsT=xT_sb[:, et, st * 128:(st + 1) * 128],
                        rhs=wv_sb[:, et, :],
                        start=(et == 0), stop=(et == ET - 1),
                    )
                nc.vector.memset(Vones[st], 1.0)
                for j in range(GH):
                    nc.vector.tensor_copy(
                        Vones[st][:, j, 0:64], psv[:, j * 64:(j + 1) * 64])

        # ---- attention + output projection ----
        with tc.tile_pool(name="attnsb", bufs=1) as at_sb, \
             tc.tile_pool(name="attnps", bufs=1, space="PSUM") as at_ps:
            for qb in range(QB):
                for p in range(NPAIR):
                    ps_av = at_ps.tile([128, 8, 128], F32, tag="pav",
                                       name="ps_av")
                    for ch in range(KTN // 2):
                        pss = at_ps.tile([128, 4, 512], F32, tag="pss",
                                         name="ps_s")
                        for i in range(4):
                            a, kl = divmod(i, 2)
                            kt = ch * 2 + kl
                            nc.tensor.matmul(
                                pss[:, i, :],
                                lhsT=KTs[p][a * 64:(a + 1) * 64,
                                            kt * 128:(kt + 1) * 128],
                                rhs=QTs[p][a * 64:(a + 1) * 64,
                                           qb * 512:(qb + 1) * 512],
                            )
                        ptt = at_sb.tile([128, 4, 512], F16, tag="ptt",
                                         bufs=2, name="ptt")
                        nc.scalar.activation(ptt, pss, AF.Exp, scale=0.125)
                        for i in range(4):
                            a, kl = divmod(i, 2)
                            kt = ch * 2 + kl
                            # ps_av rows a=0/a=1 each occupy one PSUM bank;
                            # start zeroes the whole 2KB zero region, so only
                            # the first write per bank starts and only the
                            # last write per bank stops.
                            for qw in range(4):
                                nc.tensor.matmul(
                                    ps_av[:, a * 4 + qw, 0:65],
                                    lhsT=ptt[:, i,
                                             qw * 128:(qw + 1) * 128],
                                    rhs=Vones[kt][:, 2 * p + a, :],
                                    start=(kt == 0 and qw == 0),
                                    stop=(kt == KTN - 1 and qw == 3),
                                )
                    for a in range(2):
                        for qw in range(4):
                            idx = a * 4 + qw
                            rec = at_sb.tile([128, 1], F32, tag="rec",
                                             bufs=2, name="rec")
                            nc.vector.reciprocal(
                                rec, ps_av[:, idx, 64:65])
                            otb = at_sb.tile([128, 64], F16, tag="otb",
                                             bufs=2, name="otb")
                            nc.vector.tensor_scalar_mul(
                                otb, ps_av[:, idx, 0:64], rec)
                            ptr = at_ps.tile([64, 128], F16, tag="ptr",
                                             name="ptr")
                            nc.tensor.transpose(ptr, otb, ident)
                            nc.vector.tensor_copy(
                                OTs[p][a * 64:(a + 1) * 64,
                                       qb * 512 + qw * 128:
                                       qb * 512 + (qw + 1) * 128],
                                ptr)
                for st in range(4 * qb, 4 * qb + 4):
                    osb = at_sb.tile([128, 1024], F32, tag="osb", bufs=2,
                                     name="osb")
                    for db in range(2):
                        pf = at_ps.tile([128, 512], F32, tag="pf", name="pf")
                        for j in range(NPAIR):
                            nc.tensor.matmul(
                                pf,
                                lhsT=OTs[j][:, st * 128:(st + 1) * 128],
                                rhs=wo_sb[:, j, db * 512:(db + 1) * 512],
                                start=(j == 0), stop=(j == NPAIR - 1),
                            )
                        nc.vector.tensor_copy(
                            osb[:, db * 512:(db + 1) * 512], pf)
                    nc.sync.dma_start(
                        out=out_d[st * 128:(st + 1) * 128, :], in_=osb)
    return nc


def _prep_core(inputs, c):
    b, g = divmod(c, 4)
    xT16 = np.ascontiguousarray(
        inputs["x"][b].T.astype(np.float16)).reshape(ET, 128, S)

    wqk_np = np.empty((ET, 128, 512), np.float16)
    bqk_np = np.empty((128, 4), np.float32)
    for p in range(2):
        h0 = 4 * g + 2 * p
        for qk, (W, bb) in enumerate(((inputs["Wq"], inputs["bq"]),
                                      (inputs["Wk"], inputs["bk"]))):
            blk = np.ascontiguousarray(
                W[h0 * 64:(h0 + 2) * 64, :].T.astype(np.float16))
            wqk_np[:, :, (2 * p + qk) * 128:(2 * p + qk + 1) * 128] = \
                blk.reshape(ET, 128, 128)
            bqk_np[:, 2 * p + qk] = bb[h0 * 64:(h0 + 2) * 64]

    wv_np = np.ascontiguousarray(
        inputs["Wv"][g * 256:(g + 1) * 256, :].T.astype(np.float16)
    ).reshape(ET, 128, 256)

    wo_np = np.empty((2, 128, 1024), np.float16)
    for p in range(2):
        h0 = 4 * g + 2 * p
        wo_np[p] = inputs["Wo"][:, h0 * 64:(h0 + 2) * 64].T

    return {
        "xT": xT16,
        "wqk": np.ascontiguousarray(wqk_np),
        "wv": wv_np,
        "wo": np.ascontiguousarray(wo_np),
        "bqk": bqk_np,
    }


def _run(inputs, trace=False):
    global _prog
    if _prog is None:
        _prog = _build()
        _prog.compile()
    nc = _prog
    in_maps = [_prep_core(inputs, c) for c in range(NCORES)]
    res = run_bass_kernel_spmd(nc, in_maps, list(range(NCORES)), trace=trace)
    # softmax rows sum to 1, so bv contributes the constant row bv @ Wo.T;
    # fold it and bo in on the host.
    const_row = (inputs["bv"].astype(np.float64)
                 @ inputs["Wo"].T.astype(np.float64)
                 + inputs["bo"]).astype(np.float32)
    outs = [r["out"] for r in res.results]
    final = np.empty((B, S, D), np.float32)
    for b in range(B):
        acc = outs[4 * b].astype(np.float32).copy()
        for i in range(1, 4):
            acc += outs[4 * b + i]
        final[b] = acc + const_row
    return final, res.exec_time_ns


def kernel(**inputs):
    return _run(inputs, trace=False)[0]
